# revision 18
# baseline (speedup 1.0000x reference)
"""Adaptive-computation-time LSTM (ACT-LSTM) on 8 TRN2 NeuronCores.

Data parallel: batch B=4096 is sharded 512/core; all weights replicated.
Everything runs in a transposed [feature, batch] layout so the recurrent
matmul needs no per-step transposes:

    gates.T[4H, B] = W_hh @ h.T + x_part.T   (PE, fp32r)
    h.T, c.T kept as [H, B] = 4 partition-chunks x [128, 512]

Early exit: ACT halting means p[n] = 0 for every step after all batch
elements have crossed the 1-eps cumulative-halt threshold. The first
UNC steps run unconditionally (they pipeline across steps); later steps
are wrapped in a nested-If cascade so once every element has halted the
rest are skipped with a single branch. The ACT-weighted sums (h_fin,
c_fin, ponder) are accumulated online, and the output projection is
applied once to h_fin at the end (sum_n p[n] == 1 makes that exact).

fp32r (TensorEngine full-rate fp32, 11 explicit mantissa bits, inputs
rounded RNE-12) is used for all matmuls. Weights/x/h0 are pre-rounded on
the host so plain HW-queue DMAs suffice; dynamic fp32r tensors (h, w,
h_fin) are rounded by the producing engine writing an fp32r-typed tile.

All tiles are allocated once and reused in place across steps — pool slot
cycling across If boundaries deadlocks the Tile scheduler.
"""

import numpy as np

import concourse.bass as bass
import concourse.tile as tile
from concourse import bacc, mybir
from concourse.bass_utils import run_bass_kernel_spmd

dt = mybir.dt
Alu = mybir.AluOpType
Act = mybir.ActivationFunctionType

NCORES = 8
B, I, H, O = 4096, 512, 512, 512
BS = B // NCORES          # 512 batch rows per core
P = 128                   # partitions
KH = H // P               # 4 partition-chunks for a 512-dim axis
G4 = 4 * H                # 2048 gate rows
THRESH = 1.0 - 0.01       # halting threshold (1 - EPS)
UNC = 3                   # unconditional (pipelined) leading steps

F32 = dt.float32
F32R = dt.float32r

LAST_EXEC_NS = None
LAST_RESULT = None


def _build(nmax: int):
    nc = bacc.Bacc("TRN2", target_bir_lowering=False, debug=False, num_devices=NCORES)

    # ---- DRAM I/O (per-core shard views; weights replicated) ----
    # fp32r-typed inputs are pre-rounded on the host (RNE, drop 12 bits)
    # [P, KH*BS] SBUF-image layouts: element [p, k*BS+b] = T[k*128+p, b]
    d_xT = nc.dram_tensor("xT", [P, KH * BS], F32R, kind="ExternalInput").ap()
    d_h0T = nc.dram_tensor("h0T", [P, KH * BS], F32R, kind="ExternalInput").ap()
    d_c0T = nc.dram_tensor("c0T", [P, KH * BS], F32, kind="ExternalInput").ap()
    # W_ih[:,1:].T / W_out.T pre-tiled on host into [128, n_m*KH*128] lhsT
    # stream order: block (m, k) at free offset (m*KH + k)*128
    d_wihL = nc.dram_tensor("wihL", [P, 16 * KH * P], F32R, kind="ExternalInput").ap()
    d_woL = nc.dram_tensor("woL", [P, 4 * KH * P], F32R, kind="ExternalInput").ap()
    d_whh = nc.dram_tensor("WhhT", [P, KH * G4], F32R, kind="ExternalInput").ap()  # W_hh.T image
    d_biasv = nc.dram_tensor("biasv", [P, 16], F32, kind="ExternalInput").ap()  # b_ih+b_hh
    d_flagv = nc.dram_tensor("flagv", [P, 16], F32, kind="ExternalInput").ap()  # W_ih[:,0]
    d_boutv = nc.dram_tensor("boutv", [P, 4], F32, kind="ExternalInput").ap()   # b_out
    d_whalt = nc.dram_tensor("whalt", [P, KH], F32R, kind="ExternalInput").ap()  # W_halt.T
    d_bhalt = nc.dram_tensor("bhalt", [1, 1], F32, kind="ExternalInput").ap()
    d_ident = nc.dram_tensor("ident", [P, P], F32R, kind="ExternalInput").ap()

    d_outT = nc.dram_tensor("outT", [P, KH * BS], F32, kind="ExternalOutput").ap()
    d_hfinT = nc.dram_tensor("hfinT", [P, KH * BS], F32, kind="ExternalOutput").ap()
    d_cfinT = nc.dram_tensor("cfinT", [P, KH * BS], F32, kind="ExternalOutput").ap()
    d_pond = nc.dram_tensor("ponder", [1, BS], F32, kind="ExternalOutput").ap()


    with tile.TileContext(nc) as tc:
        with (
            tc.tile_pool(name="wts", bufs=1) as wts,
            tc.tile_pool(name="wl", bufs=2) as wl,   # streamed lhsT tiles (outside Ifs only)
            tc.tile_pool(name="pps", bufs=1, space="PSUM") as pps,
        ):
            # ---------- persistent PSUM tiles (8 banks total) ----------
            gps = [pps.tile([P, 2 * BS], F32, tag=f"gps{j}", name=f"gps{j}")
                   for j in range(3)]
            ps_h = pps.tile([1, BS], F32, tag="ps_h")
            ps_w = pps.tile([P, BS], F32, tag="ps_w")

            # ---------- load weights & state ----------
            ident = wts.tile([P, P], F32R, tag="ident")
            nc.sync.dma_start(ident[:], d_ident)
            # warm the PE clock (HAM) with cheap matmuls while inputs stream
            for i in range(48):
                nc.tensor.matmul(ps_w[:, 0:P], ident[:], ident[:],
                                 start=True, stop=True)
            xt = wts.tile([P, KH * BS], F32R, tag="xt")
            for j in range(2):
                hw = KH * BS // 2
                nc.sync.dma_start(xt[:, j * hw:(j + 1) * hw],
                                  d_xT[:, j * hw:(j + 1) * hw])
            biasv = wts.tile([P, 16], F32, tag="biasv")
            nc.sync.dma_start(biasv[:], d_biasv)
            flagv = wts.tile([P, 16], F32, tag="flagv")
            nc.sync.dma_start(flagv[:], d_flagv)
            boutv = wts.tile([P, 4], F32, tag="boutv")
            nc.sync.dma_start(boutv[:], d_boutv)
            whalt = wts.tile([P, KH], F32R, tag="whalt")
            nc.sync.dma_start(whalt[:], d_whalt)
            bhalt = wts.tile([1, 1], F32, tag="bhalt")
            nc.sync.dma_start(bhalt[:], d_bhalt)
            whh = wts.tile([P, KH * G4], F32R, tag="whh")
            h = wts.tile([P, KH * BS], F32R, tag="h")
            c = wts.tile([P, KH * BS], F32, tag="c")

            # ones row for the broadcast rank-1 matmul (Memset can't write
            # fp32r; round via a DVE copy)
            ones_f = wts.tile([1, P], F32, tag="ones_f")
            nc.vector.memset(ones_f[:], 1.0)
            ones = wts.tile([1, P], F32R, tag="ones")
            nc.vector.tensor_copy(ones[:], ones_f[:])

            # persistent accumulators / scan state / step temps
            hacc = wts.tile([P, KH * BS], F32, tag="hacc")
            cacc = wts.tile([P, KH * BS], F32, tag="cacc")
            nc.vector.memset(hacc[:], 0.0)
            nc.gpsimd.memset(cacc[:], 0.0)
            cum = wts.tile([1, BS], F32, tag="cum")
            nc.vector.memset(cum[:], 0.0)
            pond = wts.tile([1, BS], F32, tag="pond")
            nc.gpsimd.memset(pond[:], 0.0)
            aA = wts.tile([1, BS], F32, tag="aA")
            nc.vector.memset(aA[:], 1.0)
            aB = wts.tile([1, BS], F32, tag="aB")

            sig_i = wts.tile([P, KH * BS], F32, tag="sig_i")
            sig_f = wts.tile([P, KH * BS], F32, tag="sig_f")
            tg = wts.tile([P, KH * BS], F32, tag="tg")
            sig_o = wts.tile([P, KH * BS], F32, tag="sig_o")
            halt = wts.tile([1, BS], F32, tag="halt")
            u1 = wts.tile([1, BS], F32, tag="u1")
            tuc = wts.tile([1, BS], F32, tag="tuc")
            w = wts.tile([1, BS], F32, tag="w")
            pc = wts.tile([1, BS], F32, tag="pc")
            wr = wts.tile([1, BS], F32R, tag="wr")
            wbc4 = wts.tile([P, KH * BS], F32, tag="wbc4")
            nact = wts.tile([1, 1], F32, tag="nact")
            nacti = wts.tile([1, 1], dt.int32, tag="nacti")
            outsb = wts.tile([P, KH * BS], F32, tag="outsb")

            # ---------- x_part = W_ih' @ x.T + (b_ih + b_hh) ----------
            # evacuate via DVE tensor_scalar (adds the per-partition bias and
            # rounds to fp32r in one op)
            xp = wts.tile([P, 16 * BS], F32R, tag="xp")
            wlt4 = None
            for g in range(8):
                ps = gps[g % 3]
                for half in range(2):
                    m = 2 * g + half
                    sl = ps[:, half * BS:(half + 1) * BS]
                    if m % 4 == 0:
                        wlt4 = wl.tile([P, 4 * KH * P], F32R, tag="wl",
                                       name=f"wih_m{m}")
                        nc.sync.dma_start(
                            wlt4[:], d_wihL[:, m * KH * P:(m + 4) * KH * P])
                    mo = (m % 4) * KH * P
                    for k in range(KH):
                        nc.tensor.matmul(
                            sl,
                            wlt4[:, mo + k * P: mo + (k + 1) * P],
                            xt[:, k * BS:(k + 1) * BS],
                            start=(k == 0), stop=(k == KH - 1),
                        )
                    nc.vector.tensor_scalar(xp[:, m * BS:(m + 1) * BS], sl,
                                            biasv[:, m:m + 1], None, Alu.add)

            # bulk weights/state stream in behind the prologue's inputs
            for k in range(KH):
                nc.sync.dma_start(whh[:, k * G4:(k + 1) * G4],
                                  d_whh[:, k * G4:(k + 1) * G4])
            for j in range(2):
                hw = KH * BS // 2
                nc.sync.dma_start(h[:, j * hw:(j + 1) * hw],
                                  d_h0T[:, j * hw:(j + 1) * hw])
                nc.sync.dma_start(c[:, j * hw:(j + 1) * hw],
                                  d_c0T[:, j * hw:(j + 1) * hw])

            gate_dst = [(sig_i, Act.Sigmoid), (sig_f, Act.Sigmoid),
                        (tg, Act.Tanh), (sig_o, Act.Sigmoid)]

            # ---------- one ACT-LSTM step ----------
            def step(n: int):
                first = n == 0
                last = n == nmax - 1
                a_cur = aA if n % 2 == 0 else aB
                a_nxt = aB if n % 2 == 0 else aA

                # gates.T = x_part.T + W_hh @ h.T ; evacuate through the
                # activation fns (step 0 adds the ACT flag column as bias)
                for g in range(8):
                    ps = gps[g % 3]
                    for half in range(2):
                        m = 2 * g + half
                        sl = ps[:, half * BS:(half + 1) * BS]
                        nc.tensor.matmul(sl, ident[:],
                                         xp[:, m * BS:(m + 1) * BS],
                                         start=True, stop=False)
                        for k in range(KH):
                            nc.tensor.matmul(
                                sl,
                                whh[:, k * G4 + m * P: k * G4 + (m + 1) * P],
                                h[:, k * BS:(k + 1) * BS],
                                start=False, stop=(k == KH - 1),
                            )
                    dst, fn = gate_dst[g // 2]
                    off = (g % 2) * 2 * BS
                    if first:
                        for half in range(2):
                            m = 2 * g + half
                            nc.scalar.activation(
                                dst[:, off + half * BS: off + (half + 1) * BS],
                                ps[:, half * BS:(half + 1) * BS], fn,
                                bias=flagv[:, m:m + 1])
                    else:
                        nc.scalar.activation(dst[:, off:off + 2 * BS], ps[:], fn)

                # LSTM cell, chunked so h[k] lands early (shortens the
                # loop-carried path into the next step's matmuls)
                for k in range(KH):
                    ck = slice(k * BS, (k + 1) * BS)
                    nc.vector.tensor_tensor(tg[:, ck], sig_i[:, ck], tg[:, ck], Alu.mult)
                    nc.vector.tensor_tensor(sig_f[:, ck], sig_f[:, ck], c[:, ck], Alu.mult)
                    nc.vector.tensor_tensor(c[:, ck], sig_f[:, ck], tg[:, ck], Alu.add)
                    nc.scalar.activation(tg[:, ck], c[:, ck], Act.Tanh)
                    nc.vector.tensor_tensor(h[:, ck], sig_o[:, ck], tg[:, ck], Alu.mult)

                # halt = sigmoid(W_halt @ h + b_halt)   [1, BS]
                for k in range(KH):
                    nc.tensor.matmul(ps_h[:], whalt[:, k:k + 1],
                                     h[:, k * BS:(k + 1) * BS],
                                     start=(k == 0), stop=(k == KH - 1))
                nc.scalar.activation(halt[:], ps_h[:], Act.Sigmoid,
                                     bias=bhalt[0:1, 0:1])

                # ACT weights:  w = a * (halt + (cum>=t) * (1 - cum_new))
                nc.vector.tensor_tensor(cum[:], cum[:], halt[:], Alu.add)
                nc.vector.tensor_scalar(u1[:], cum[:], -1.0, 1.0, Alu.mult, Alu.add)
                cf_thresh = -3.0e38 if last else THRESH
                nc.vector.scalar_tensor_tensor(tuc[:], cum[:], cf_thresh, u1[:],
                                               Alu.is_ge, Alu.mult)
                nc.vector.scalar_tensor_tensor(w[:], tuc[:], 0.0, halt[:],
                                               Alu.bypass, Alu.add)
                nc.vector.scalar_tensor_tensor(w[:], w[:], 0.0, a_cur[:],
                                               Alu.bypass, Alu.mult)
                nc.vector.tensor_copy(wr[:], w[:])

                # ponder += a + (cum>=t)*w   (off the w critical path)
                nc.vector.scalar_tensor_tensor(pc[:], cum[:], cf_thresh, w[:],
                                               Alu.is_ge, Alu.mult)
                nc.vector.scalar_tensor_tensor(pc[:], pc[:], 0.0, a_cur[:],
                                               Alu.bypass, Alu.add)
                nc.gpsimd.tensor_tensor(pond[:], pond[:], pc[:], Alu.add)

                # broadcast w across partitions via rank-1 matmul, then
                # replicate along the 4 H-chunks (flat APs keep DVE at rate)
                nc.tensor.matmul(ps_w[:], ones[:], wr[:], start=True, stop=True)
                for k in range(KH):
                    nc.scalar.copy(wbc4[:, k * BS:(k + 1) * BS], ps_w[:])

                # weighted accumulation (flat [128, 2048] ops)
                # tg/sig_i are dead after the cell update; reuse as temps
                nc.vector.tensor_tensor(tg[:], h[:], wbc4[:], Alu.mult)
                nc.vector.tensor_tensor(hacc[:], hacc[:], tg[:], Alu.add)
                nc.vector.tensor_tensor(sig_i[:], c[:], wbc4[:], Alu.mult)
                nc.gpsimd.tensor_tensor(cacc[:], cacc[:], sig_i[:], Alu.add)

                if last:
                    return
                # next-step active mask + active count (for the early-exit If)
                nc.vector.tensor_scalar(a_nxt[:], cum[:], THRESH, 0.0,
                                        Alu.is_lt, Alu.add, accum_out=nact[:])
                nc.vector.tensor_copy(nacti[:], nact[:])

            def run_steps(n: int):
                step(n)
                if n + 1 >= nmax:
                    return
                if n + 1 < UNC:
                    run_steps(n + 1)
                    return
                v = nc.values_load(nacti[:])
                with tc.If(v > 0):
                    run_steps(n + 1)

            run_steps(0)

            # ---------- epilogue: out.T = W_out @ h_fin + b_out ----------
            for j in range(2):
                hw = KH * BS // 2
                nc.sync.dma_start(d_hfinT[:, j * hw:(j + 1) * hw],
                                  hacc[:, j * hw:(j + 1) * hw])
                nc.sync.dma_start(d_cfinT[:, j * hw:(j + 1) * hw],
                                  cacc[:, j * hw:(j + 1) * hw])
            nc.sync.dma_start(d_pond, pond[:])
            hacc_r = wts.tile([P, KH * BS], F32R, tag="hacc_r")
            nc.vector.tensor_copy(hacc_r[:], hacc[:])
            wlo = wl.tile([P, 4 * KH * P], F32R, tag="wl", name="wo_all")
            nc.sync.dma_start(wlo[:], d_woL[:])
            for g in range(2):
                ps = gps[g % 3]
                for half in range(2):
                    m = 2 * g + half
                    sl = ps[:, half * BS:(half + 1) * BS]
                    mo = m * KH * P
                    for k in range(KH):
                        nc.tensor.matmul(
                            sl,
                            wlo[:, mo + k * P: mo + (k + 1) * P],
                            hacc_r[:, k * BS:(k + 1) * BS],
                            start=(k == 0), stop=(k == KH - 1),
                        )
                    nc.scalar.activation(outsb[:, m * BS:(m + 1) * BS], sl,
                                         Act.Identity, bias=boutv[:, m:m + 1])

            for j in range(2):
                hw = KH * BS // 2
                nc.sync.dma_start(d_outT[:, j * hw:(j + 1) * hw],
                                  outsb[:, j * hw:(j + 1) * hw])

    nc.compile()
    return nc


_CACHE: dict = {}


def _get_nc(nmax: int):
    if nmax not in _CACHE:
        _CACHE[nmax] = _build(nmax)
    return _CACHE[nmax]


def _r(x: np.ndarray) -> np.ndarray:
    """Round fp32 -> fp32r bits (RNE dropping the low 12 mantissa bits)."""
    shape = x.shape
    u = np.ascontiguousarray(x, np.float32).view(np.uint32).astype(np.uint64)
    mask = np.uint64(0xFFF)
    half = np.uint64(0x800)
    low = u & mask
    base = u & ~mask
    lsb = (u >> np.uint64(12)) & np.uint64(1)
    add = (low > half) | ((low == half) & (lsb == np.uint64(1)))
    out = base + np.where(add, np.uint64(0x1000), np.uint64(0))
    return (out & np.uint64(0xFFFFFFFF)).astype(np.uint32).view(np.float32).reshape(shape)


def _img(T: np.ndarray) -> np.ndarray:
    """[KH*128, N] -> SBUF image [128, KH*N]: img[p, k*N+n] = T[k*128+p, n]."""
    kh = T.shape[0] // P
    return np.ascontiguousarray(
        T.reshape(kh, P, T.shape[1]).transpose(1, 0, 2).reshape(P, kh * T.shape[1]))


def _unimg(img: np.ndarray, kh: int) -> np.ndarray:
    """Inverse of _img: [128, kh*N] -> [kh*128, N]."""
    N = img.shape[1] // kh
    return img.reshape(P, kh, N).transpose(1, 0, 2).reshape(kh * P, N)


def _lhsT_stream(WT: np.ndarray, n_m: int) -> np.ndarray:
    """[K, M] -> [128, n_m*KH*128] with block (m, k) at offset (m*KH+k)*128.

    Element [p, (m*KH+k)*128 + q] = WT[k*128 + p, m*128 + q].
    """
    K, M = WT.shape
    kh = K // P
    assert M == n_m * P
    t = WT.reshape(kh, P, n_m, P).transpose(1, 2, 0, 3).reshape(P, n_m * kh * P)
    return np.ascontiguousarray(t)


def kernel(x, h0, c0, W_ih, b_ih, W_hh, b_hh, W_halt, b_halt, W_out, b_out,
           max_steps):
    global LAST_EXEC_NS, LAST_RESULT
    f = np.float32
    x = np.asarray(x, f)
    h0 = np.asarray(h0, f)
    c0 = np.asarray(c0, f)
    W_ih = np.asarray(W_ih, f)
    b_ih = np.asarray(b_ih, f)
    W_hh = np.asarray(W_hh, f)
    b_hh = np.asarray(b_hh, f)
    W_halt = np.asarray(W_halt, f)
    b_halt = np.asarray(b_halt, f)
    W_out = np.asarray(W_out, f)
    b_out = np.asarray(b_out, f)
    nmax = int(max_steps)

    shared = {
        "wihL": _r(_lhsT_stream(np.ascontiguousarray(W_ih[:, 1:].T), 16)),
        "woL": _r(_lhsT_stream(np.ascontiguousarray(W_out.T), 4)),
        "WhhT": _r(_img(W_hh.T)),
        "biasv": np.ascontiguousarray((b_ih + b_hh).reshape(16, P).T),
        "flagv": np.ascontiguousarray(W_ih[:, 0].reshape(16, P).T),
        "boutv": np.ascontiguousarray(b_out.reshape(4, P).T),
        "whalt": _r(np.ascontiguousarray(W_halt[0].reshape(KH, P).T)),
        "bhalt": b_halt.reshape(1, 1),
        "ident": np.eye(P, dtype=f),
    }
    in_maps = []
    for i in range(NCORES):
        s = slice(i * BS, (i + 1) * BS)
        in_maps.append({
            "xT": _r(_img(x[s].T)),
            "h0T": _r(_img(h0[s].T)),
            "c0T": _img(c0[s].T),
            **shared,
        })

    nc = _get_nc(nmax)
    res = run_bass_kernel_spmd(nc, in_maps, core_ids=list(range(NCORES)))
    LAST_EXEC_NS = res.exec_time_ns
    LAST_RESULT = res

    outs, hfs, cfs, ponds = [], [], [], []
    for i in range(NCORES):
        r = res.results[i]
        outs.append(_unimg(r["outT"], KH).T)
        hfs.append(_unimg(r["hfinT"], KH).T)
        cfs.append(_unimg(r["cfinT"], KH).T)
        ponds.append(r["ponder"][0])
    output = np.ascontiguousarray(np.concatenate(outs, 0))
    h_fin = np.ascontiguousarray(np.concatenate(hfs, 0))
    c_fin = np.ascontiguousarray(np.concatenate(cfs, 0))
    ponder = np.ascontiguousarray(np.concatenate(ponds, 0))
    return output, h_fin, c_fin, ponder


# revision 20
# speedup vs baseline: 1.0639x; 1.0639x over previous
"""Adaptive-computation-time LSTM (ACT-LSTM) on 8 TRN2 NeuronCores.

Data parallel: batch B=4096 is sharded 512/core; all weights replicated.
Everything runs in a transposed [feature, batch] layout so the recurrent
matmul needs no per-step transposes:

    gates.T[4H, B] = W_hh @ h.T + x_part.T   (PE, fp32r)
    h.T, c.T kept as [H, B] = 4 partition-chunks x [128, 512]

Early exit: ACT halting means p[n] = 0 for every step after all batch
elements have crossed the 1-eps cumulative-halt threshold. The first
UNC steps run unconditionally (they pipeline across steps); later steps
are wrapped in a nested-If cascade so once every element has halted the
rest are skipped with a single branch. The ACT-weighted sums (h_fin,
c_fin, ponder) are accumulated online, and the output projection is
applied once to h_fin at the end (sum_n p[n] == 1 makes that exact).

fp32r (TensorEngine full-rate fp32, 11 explicit mantissa bits, inputs
rounded RNE-12) is used for all matmuls. Weights/x/h0 are pre-rounded on
the host so plain HW-queue DMAs suffice; dynamic fp32r tensors (h, w,
h_fin) are rounded by the producing engine writing an fp32r-typed tile.

All tiles are allocated once and reused in place across steps — pool slot
cycling across If boundaries deadlocks the Tile scheduler.
"""

import numpy as np

import concourse.bass as bass
import concourse.tile as tile
from concourse import bacc, mybir
from concourse.bass_utils import run_bass_kernel_spmd

dt = mybir.dt
Alu = mybir.AluOpType
Act = mybir.ActivationFunctionType

NCORES = 8
B, I, H, O = 4096, 512, 512, 512
BS = B // NCORES          # 512 batch rows per core
P = 128                   # partitions
KH = H // P               # 4 partition-chunks for a 512-dim axis
G4 = 4 * H                # 2048 gate rows
THRESH = 1.0 - 0.01       # halting threshold (1 - EPS)
UNC = 3                   # unconditional (pipelined) leading steps

F32 = dt.float32
F32R = dt.float32r

LAST_EXEC_NS = None
LAST_RESULT = None


def _build(nmax: int):
    nc = bacc.Bacc("TRN2", target_bir_lowering=False, debug=False, num_devices=NCORES)

    # ---- DRAM I/O (per-core shard views; weights replicated) ----
    # fp32r-typed inputs are pre-rounded on the host (RNE, drop 12 bits)
    # [P, KH*BS] SBUF-image layouts: element [p, k*BS+b] = T[k*128+p, b]
    d_xT = nc.dram_tensor("xT", [P, KH * BS], F32R, kind="ExternalInput").ap()
    d_h0T = nc.dram_tensor("h0T", [P, KH * BS], F32R, kind="ExternalInput").ap()
    d_c0T = nc.dram_tensor("c0T", [P, KH * BS], F32, kind="ExternalInput").ap()
    # W_ih[:,1:].T / W_out.T pre-tiled on host into [128, n_m*KH*128] lhsT
    # stream order: block (m, k) at free offset (m*KH + k)*128
    d_wihL = nc.dram_tensor("wihL", [P, 16 * KH * P], F32R, kind="ExternalInput").ap()
    d_woL = nc.dram_tensor("woL", [P, 4 * KH * P], F32R, kind="ExternalInput").ap()
    d_whh = nc.dram_tensor("WhhT", [P, KH * G4], F32R, kind="ExternalInput").ap()  # W_hh.T image
    d_biasv = nc.dram_tensor("biasv", [P, 16], F32, kind="ExternalInput").ap()  # b_ih+b_hh
    d_flagv = nc.dram_tensor("flagv", [P, 16], F32, kind="ExternalInput").ap()  # W_ih[:,0]
    d_boutv = nc.dram_tensor("boutv", [P, 4], F32, kind="ExternalInput").ap()   # b_out
    d_whalt = nc.dram_tensor("whalt", [P, KH], F32R, kind="ExternalInput").ap()  # W_halt.T
    d_bhalt = nc.dram_tensor("bhalt", [1, 1], F32, kind="ExternalInput").ap()
    d_ident = nc.dram_tensor("ident", [P, P], F32R, kind="ExternalInput").ap()

    d_outT = nc.dram_tensor("outT", [P, KH * BS], F32, kind="ExternalOutput").ap()
    d_hfinT = nc.dram_tensor("hfinT", [P, KH * BS], F32, kind="ExternalOutput").ap()
    d_cfinT = nc.dram_tensor("cfinT", [P, KH * BS], F32, kind="ExternalOutput").ap()
    d_pond = nc.dram_tensor("ponder", [1, BS], F32, kind="ExternalOutput").ap()


    with tile.TileContext(nc) as tc:
        with (
            tc.tile_pool(name="wts", bufs=1) as wts,
            tc.tile_pool(name="wl", bufs=2) as wl,   # streamed lhsT tiles (outside Ifs only)
            tc.tile_pool(name="pps", bufs=1, space="PSUM") as pps,
        ):
            # ---------- persistent PSUM tiles (8 banks total) ----------
            gps = [pps.tile([P, 2 * BS], F32, tag=f"gps{j}", name=f"gps{j}")
                   for j in range(3)]
            ps_h = pps.tile([1, BS], F32, tag="ps_h")
            ps_w = pps.tile([P, BS], F32, tag="ps_w")

            # ---------- load weights & state ----------
            ident = wts.tile([P, P], F32R, tag="ident")
            nc.sync.dma_start(ident[:], d_ident)
            # warm the PE clock (HAM) with cheap matmuls while inputs stream
            for i in range(48):
                nc.tensor.matmul(ps_w[:, 0:P], ident[:], ident[:],
                                 start=True, stop=True)
            xt = wts.tile([P, KH * BS], F32R, tag="xt")
            for j in range(2):
                hw = KH * BS // 2
                nc.sync.dma_start(xt[:, j * hw:(j + 1) * hw],
                                  d_xT[:, j * hw:(j + 1) * hw])
            biasv = wts.tile([P, 16], F32, tag="biasv")
            nc.sync.dma_start(biasv[:], d_biasv)
            flagv = wts.tile([P, 16], F32, tag="flagv")
            nc.sync.dma_start(flagv[:], d_flagv)
            boutv = wts.tile([P, 4], F32, tag="boutv")
            nc.sync.dma_start(boutv[:], d_boutv)
            whalt = wts.tile([P, KH], F32R, tag="whalt")
            nc.sync.dma_start(whalt[:], d_whalt)
            bhalt = wts.tile([1, 1], F32, tag="bhalt")
            nc.sync.dma_start(bhalt[:], d_bhalt)
            whh = wts.tile([P, KH * G4], F32R, tag="whh")
            h = wts.tile([P, KH * BS], F32R, tag="h")
            c = wts.tile([P, KH * BS], F32, tag="c")

            # ones row for the broadcast rank-1 matmul (Memset can't write
            # fp32r; round via a DVE copy)
            ones_f = wts.tile([1, P], F32, tag="ones_f")
            nc.vector.memset(ones_f[:], 1.0)
            ones = wts.tile([1, P], F32R, tag="ones")
            nc.vector.tensor_copy(ones[:], ones_f[:])

            # persistent accumulators / scan state / step temps
            hacc = wts.tile([P, KH * BS], F32, tag="hacc")
            cacc = wts.tile([P, KH * BS], F32, tag="cacc")
            nc.vector.memset(hacc[:], 0.0)
            nc.gpsimd.memset(cacc[:], 0.0)
            cum = wts.tile([1, BS], F32, tag="cum")
            nc.vector.memset(cum[:], 0.0)
            pond = wts.tile([1, BS], F32, tag="pond")
            nc.gpsimd.memset(pond[:], 0.0)
            aA = wts.tile([1, BS], F32, tag="aA")
            nc.vector.memset(aA[:], 1.0)
            aB = wts.tile([1, BS], F32, tag="aB")

            sig_i = wts.tile([P, KH * BS], F32, tag="sig_i")
            sig_f = wts.tile([P, KH * BS], F32, tag="sig_f")
            tg = wts.tile([P, KH * BS], F32, tag="tg")
            sig_o = wts.tile([P, KH * BS], F32, tag="sig_o")
            halt = wts.tile([1, BS], F32, tag="halt")
            u1 = wts.tile([1, BS], F32, tag="u1")
            tuc = wts.tile([1, BS], F32, tag="tuc")
            w = wts.tile([1, BS], F32, tag="w")
            pc = wts.tile([1, BS], F32, tag="pc")
            wr = wts.tile([1, BS], F32R, tag="wr")
            wbc4 = wts.tile([P, KH * BS], F32, tag="wbc4")
            hacc_r32 = wts.tile([P, KH * BS], F32, tag="hacc_r32")
            nact = wts.tile([1, 1], F32, tag="nact")
            nacti = wts.tile([1, 1], dt.int32, tag="nacti")
            outsb = wts.tile([P, KH * BS], F32, tag="outsb")

            # ---------- x_part = W_ih' @ x.T + (b_ih + b_hh) ----------
            # evacuate via DVE tensor_scalar (adds the per-partition bias and
            # rounds to fp32r in one op)
            xp = wts.tile([P, 16 * BS], F32R, tag="xp")
            wlt4 = None
            for g in range(8):
                ps = gps[g % 3]
                for half in range(2):
                    m = 2 * g + half
                    sl = ps[:, half * BS:(half + 1) * BS]
                    if m % 4 == 0:
                        wlt4 = wl.tile([P, 4 * KH * P], F32R, tag="wl",
                                       name=f"wih_m{m}")
                        nc.sync.dma_start(
                            wlt4[:], d_wihL[:, m * KH * P:(m + 4) * KH * P])
                    mo = (m % 4) * KH * P
                    for k in range(KH):
                        nc.tensor.matmul(
                            sl,
                            wlt4[:, mo + k * P: mo + (k + 1) * P],
                            xt[:, k * BS:(k + 1) * BS],
                            start=(k == 0), stop=(k == KH - 1),
                        )
                    nc.vector.tensor_scalar(xp[:, m * BS:(m + 1) * BS], sl,
                                            biasv[:, m:m + 1], None, Alu.add)

            # bulk weights/state stream in behind the prologue's inputs
            for k in range(KH):
                nc.sync.dma_start(whh[:, k * G4:(k + 1) * G4],
                                  d_whh[:, k * G4:(k + 1) * G4])
            for j in range(2):
                hw = KH * BS // 2
                nc.sync.dma_start(h[:, j * hw:(j + 1) * hw],
                                  d_h0T[:, j * hw:(j + 1) * hw])
                nc.sync.dma_start(c[:, j * hw:(j + 1) * hw],
                                  d_c0T[:, j * hw:(j + 1) * hw])

            gate_dst = [(sig_i, Act.Sigmoid), (sig_f, Act.Sigmoid),
                        (tg, Act.Tanh), (sig_o, Act.Sigmoid)]

            # ---------- one ACT-LSTM step ----------
            def make_acc(n):
                """Emit step n's w-broadcast + weighted accumulation.

                Deferred into the next step's matmul burst (or the epilogue)
                so the in-order PE queue never stalls on the small-vector
                chain. Reads h/c before the next cell update rewrites them
                (Tile's WAR deps order that)."""
                def emit():
                    nc.tensor.matmul(ps_w[:], ones[:], wr[:], start=True, stop=True)
                    for k in range(KH):
                        ck = slice(k * BS, (k + 1) * BS)
                        nc.scalar.copy(wbc4[:, ck], ps_w[:])
                        # outsb is idle until the epilogue; use as the c temp,
                        # then fold h into wbc4 in place (dead afterwards)
                        nc.vector.tensor_tensor(outsb[:, ck], c[:, ck],
                                                wbc4[:, ck], Alu.mult)
                        nc.gpsimd.tensor_tensor(cacc[:, ck], cacc[:, ck],
                                                outsb[:, ck], Alu.add)
                        nc.vector.tensor_tensor(wbc4[:, ck], h[:, ck].bitcast(F32),
                                                wbc4[:, ck], Alu.mult)
                        nc.vector.tensor_tensor(hacc[:, ck], hacc[:, ck],
                                                wbc4[:, ck], Alu.add)
                return emit

            def step(n: int, emit_prev_acc):
                first = n == 0
                last = n == nmax - 1
                a_cur = aA if n % 2 == 0 else aB
                a_nxt = aB if n % 2 == 0 else aA

                # front-run the first 3 groups' x_part injections (no h dep),
                # then the halt matvec of the PREVIOUS h... the halt for THIS
                # step comes after the cell below.
                for g in range(3):
                    ps = gps[g]
                    for half in range(2):
                        m = 2 * g + half
                        nc.tensor.matmul(ps[:, half * BS:(half + 1) * BS],
                                         ident[:], xp[:, m * BS:(m + 1) * BS],
                                         start=True, stop=False)

                # gates.T = x_part.T + W_hh @ h.T ; evacuate through the
                # activation fns (step 0 adds the ACT flag column as bias)
                for g in range(8):
                    ps = gps[g % 3]
                    for half in range(2):
                        m = 2 * g + half
                        sl = ps[:, half * BS:(half + 1) * BS]
                        if g >= 3:
                            nc.tensor.matmul(sl, ident[:],
                                             xp[:, m * BS:(m + 1) * BS],
                                             start=True, stop=False)
                        for k in range(KH):
                            nc.tensor.matmul(
                                sl,
                                whh[:, k * G4 + m * P: k * G4 + (m + 1) * P],
                                h[:, k * BS:(k + 1) * BS],
                                start=False, stop=(k == KH - 1),
                            )
                    dst, fn = gate_dst[g // 2]
                    off = (g % 2) * 2 * BS
                    if first:
                        for half in range(2):
                            m = 2 * g + half
                            nc.scalar.activation(
                                dst[:, off + half * BS: off + (half + 1) * BS],
                                ps[:, half * BS:(half + 1) * BS], fn,
                                bias=flagv[:, m:m + 1])
                    else:
                        nc.scalar.activation(dst[:, off:off + 2 * BS], ps[:], fn)
                    if g == 1 and emit_prev_acc is not None:
                        emit_prev_acc()

                # LSTM cell, chunked so h[k] lands early (shortens the
                # loop-carried path into the next step's matmuls)
                for k in range(KH):
                    ck = slice(k * BS, (k + 1) * BS)
                    nc.vector.tensor_tensor(tg[:, ck], sig_i[:, ck], tg[:, ck], Alu.mult)
                    nc.vector.tensor_tensor(sig_f[:, ck], sig_f[:, ck], c[:, ck], Alu.mult)
                    nc.vector.tensor_tensor(c[:, ck], sig_f[:, ck], tg[:, ck], Alu.add)
                    nc.scalar.activation(tg[:, ck], c[:, ck], Act.Tanh)
                    nc.vector.tensor_tensor(h[:, ck], sig_o[:, ck], tg[:, ck], Alu.mult)
                    # halt matvec chunk as soon as h[k] exists
                    nc.tensor.matmul(ps_h[:], whalt[:, k:k + 1], h[:, ck],
                                     start=(k == 0), stop=(k == KH - 1))
                nc.scalar.activation(halt[:], ps_h[:], Act.Sigmoid,
                                     bias=bhalt[0:1, 0:1])

                # ACT weights:  w = a * (halt + (cum>=t) * (1 - cum_new))
                nc.vector.tensor_tensor(cum[:], cum[:], halt[:], Alu.add)
                nc.vector.tensor_scalar(u1[:], cum[:], -1.0, 1.0, Alu.mult, Alu.add)
                cf_thresh = -3.0e38 if last else THRESH
                nc.vector.scalar_tensor_tensor(tuc[:], cum[:], cf_thresh, u1[:],
                                               Alu.is_ge, Alu.mult)
                nc.vector.scalar_tensor_tensor(w[:], tuc[:], 0.0, halt[:],
                                               Alu.bypass, Alu.add)
                nc.vector.scalar_tensor_tensor(w[:], w[:], 0.0, a_cur[:],
                                               Alu.bypass, Alu.mult)
                nc.vector.tensor_copy(wr[:], w[:])

                # ponder += a + (cum>=t)*w   (off the w critical path)
                nc.vector.scalar_tensor_tensor(pc[:], cum[:], cf_thresh, w[:],
                                               Alu.is_ge, Alu.mult)
                nc.vector.scalar_tensor_tensor(pc[:], pc[:], 0.0, a_cur[:],
                                               Alu.bypass, Alu.add)
                nc.gpsimd.tensor_tensor(pond[:], pond[:], pc[:], Alu.add)

                if last:
                    return
                # next-step active mask + active count (for the early-exit If)
                nc.vector.tensor_scalar(a_nxt[:], cum[:], THRESH, 0.0,
                                        Alu.is_lt, Alu.add, accum_out=nact[:])
                nc.vector.tensor_copy(nacti[:], nact[:])

            def run_steps(n: int, emit_prev_acc):
                step(n, emit_prev_acc)
                acc_self = make_acc(n)
                if n + 1 >= nmax:
                    acc_self()
                    return
                if n + 1 < UNC:
                    run_steps(n + 1, acc_self)
                    return
                v = nc.values_load(nacti[:])
                with tc.If(v > 0) as cmp:
                    run_steps(n + 1, acc_self)
                with cmp.Else():
                    acc_self()

            run_steps(0, None)

            # ---------- epilogue: out.T = W_out @ h_fin + b_out ----------
            for j in range(2):
                hw = KH * BS // 2
                nc.sync.dma_start(d_hfinT[:, j * hw:(j + 1) * hw],
                                  hacc[:, j * hw:(j + 1) * hw])
                nc.sync.dma_start(d_cfinT[:, j * hw:(j + 1) * hw],
                                  cacc[:, j * hw:(j + 1) * hw])
            nc.sync.dma_start(d_pond, pond[:])
            # chunked fp32r rounding of h_fin so the projection can start
            # as soon as chunk 0 is ready
            hacc_r = hacc_r32[:].bitcast(F32R)
            for k in range(KH):
                ck = slice(k * BS, (k + 1) * BS)
                nc.vector.tensor_copy(hacc_r32[:, ck].bitcast(F32R), hacc[:, ck])
            wlo = wl.tile([P, 4 * KH * P], F32R, tag="wl", name="wo_all")
            nc.sync.dma_start(wlo[:], d_woL[:])
            for g in range(2):
                ps = gps[g % 3]
                for half in range(2):
                    m = 2 * g + half
                    sl = ps[:, half * BS:(half + 1) * BS]
                    mo = m * KH * P
                    for k in range(KH):
                        nc.tensor.matmul(
                            sl,
                            wlo[:, mo + k * P: mo + (k + 1) * P],
                            hacc_r32[:, k * BS:(k + 1) * BS].bitcast(F32R),
                            start=(k == 0), stop=(k == KH - 1),
                        )
                    nc.scalar.activation(outsb[:, m * BS:(m + 1) * BS], sl,
                                         Act.Identity, bias=boutv[:, m:m + 1])

            for j in range(2):
                hw = KH * BS // 2
                nc.sync.dma_start(d_outT[:, j * hw:(j + 1) * hw],
                                  outsb[:, j * hw:(j + 1) * hw])

    nc.compile()
    return nc


_CACHE: dict = {}


def _get_nc(nmax: int):
    if nmax not in _CACHE:
        _CACHE[nmax] = _build(nmax)
    return _CACHE[nmax]


def _r(x: np.ndarray) -> np.ndarray:
    """Round fp32 -> fp32r bits (RNE dropping the low 12 mantissa bits)."""
    shape = x.shape
    u = np.ascontiguousarray(x, np.float32).view(np.uint32).astype(np.uint64)
    mask = np.uint64(0xFFF)
    half = np.uint64(0x800)
    low = u & mask
    base = u & ~mask
    lsb = (u >> np.uint64(12)) & np.uint64(1)
    add = (low > half) | ((low == half) & (lsb == np.uint64(1)))
    out = base + np.where(add, np.uint64(0x1000), np.uint64(0))
    return (out & np.uint64(0xFFFFFFFF)).astype(np.uint32).view(np.float32).reshape(shape)


def _img(T: np.ndarray) -> np.ndarray:
    """[KH*128, N] -> SBUF image [128, KH*N]: img[p, k*N+n] = T[k*128+p, n]."""
    kh = T.shape[0] // P
    return np.ascontiguousarray(
        T.reshape(kh, P, T.shape[1]).transpose(1, 0, 2).reshape(P, kh * T.shape[1]))


def _unimg(img: np.ndarray, kh: int) -> np.ndarray:
    """Inverse of _img: [128, kh*N] -> [kh*128, N]."""
    N = img.shape[1] // kh
    return img.reshape(P, kh, N).transpose(1, 0, 2).reshape(kh * P, N)


def _lhsT_stream(WT: np.ndarray, n_m: int) -> np.ndarray:
    """[K, M] -> [128, n_m*KH*128] with block (m, k) at offset (m*KH+k)*128.

    Element [p, (m*KH+k)*128 + q] = WT[k*128 + p, m*128 + q].
    """
    K, M = WT.shape
    kh = K // P
    assert M == n_m * P
    t = WT.reshape(kh, P, n_m, P).transpose(1, 2, 0, 3).reshape(P, n_m * kh * P)
    return np.ascontiguousarray(t)


def kernel(x, h0, c0, W_ih, b_ih, W_hh, b_hh, W_halt, b_halt, W_out, b_out,
           max_steps):
    global LAST_EXEC_NS, LAST_RESULT
    f = np.float32
    x = np.asarray(x, f)
    h0 = np.asarray(h0, f)
    c0 = np.asarray(c0, f)
    W_ih = np.asarray(W_ih, f)
    b_ih = np.asarray(b_ih, f)
    W_hh = np.asarray(W_hh, f)
    b_hh = np.asarray(b_hh, f)
    W_halt = np.asarray(W_halt, f)
    b_halt = np.asarray(b_halt, f)
    W_out = np.asarray(W_out, f)
    b_out = np.asarray(b_out, f)
    nmax = int(max_steps)

    shared = {
        "wihL": _r(_lhsT_stream(np.ascontiguousarray(W_ih[:, 1:].T), 16)),
        "woL": _r(_lhsT_stream(np.ascontiguousarray(W_out.T), 4)),
        "WhhT": _r(_img(W_hh.T)),
        "biasv": np.ascontiguousarray((b_ih + b_hh).reshape(16, P).T),
        "flagv": np.ascontiguousarray(W_ih[:, 0].reshape(16, P).T),
        "boutv": np.ascontiguousarray(b_out.reshape(4, P).T),
        "whalt": _r(np.ascontiguousarray(W_halt[0].reshape(KH, P).T)),
        "bhalt": b_halt.reshape(1, 1),
        "ident": np.eye(P, dtype=f),
    }
    in_maps = []
    for i in range(NCORES):
        s = slice(i * BS, (i + 1) * BS)
        in_maps.append({
            "xT": _r(_img(x[s].T)),
            "h0T": _r(_img(h0[s].T)),
            "c0T": _img(c0[s].T),
            **shared,
        })

    nc = _get_nc(nmax)
    res = run_bass_kernel_spmd(nc, in_maps, core_ids=list(range(NCORES)))
    LAST_EXEC_NS = res.exec_time_ns
    LAST_RESULT = res

    outs, hfs, cfs, ponds = [], [], [], []
    for i in range(NCORES):
        r = res.results[i]
        outs.append(_unimg(r["outT"], KH).T)
        hfs.append(_unimg(r["hfinT"], KH).T)
        cfs.append(_unimg(r["cfinT"], KH).T)
        ponds.append(r["ponder"][0])
    output = np.ascontiguousarray(np.concatenate(outs, 0))
    h_fin = np.ascontiguousarray(np.concatenate(hfs, 0))
    c_fin = np.ascontiguousarray(np.concatenate(cfs, 0))
    ponder = np.ascontiguousarray(np.concatenate(ponds, 0))
    return output, h_fin, c_fin, ponder


# revision 21
# speedup vs baseline: 1.0792x; 1.0143x over previous
"""Adaptive-computation-time LSTM (ACT-LSTM) on 8 TRN2 NeuronCores.

Data parallel: batch B=4096 is sharded 512/core; all weights replicated.
Everything runs in a transposed [feature, batch] layout so the recurrent
matmul needs no per-step transposes:

    gates.T[4H, B] = W_hh @ h.T + x_part.T   (PE, fp32r)
    h.T, c.T kept as [H, B] = 4 partition-chunks x [128, 512]

Early exit: ACT halting means p[n] = 0 for every step after all batch
elements have crossed the 1-eps cumulative-halt threshold. The first
UNC steps run unconditionally (they pipeline across steps); later steps
are wrapped in a nested-If cascade so once every element has halted the
rest are skipped with a single branch. The ACT-weighted sums (h_fin,
c_fin, ponder) are accumulated online, and the output projection is
applied once to h_fin at the end (sum_n p[n] == 1 makes that exact).

fp32r (TensorEngine full-rate fp32, 11 explicit mantissa bits, inputs
rounded RNE-12) is used for all matmuls. Weights/x/h0 are pre-rounded on
the host so plain HW-queue DMAs suffice; dynamic fp32r tensors (h, w,
h_fin) are rounded by the producing engine writing an fp32r-typed tile.

All tiles are allocated once and reused in place across steps — pool slot
cycling across If boundaries deadlocks the Tile scheduler.
"""

import numpy as np

import concourse.bass as bass
import concourse.tile as tile
from concourse import bacc, mybir
from concourse.bass_utils import run_bass_kernel_spmd

dt = mybir.dt
Alu = mybir.AluOpType
Act = mybir.ActivationFunctionType

NCORES = 8
B, I, H, O = 4096, 512, 512, 512
BS = B // NCORES          # 512 batch rows per core
P = 128                   # partitions
KH = H // P               # 4 partition-chunks for a 512-dim axis
G4 = 4 * H                # 2048 gate rows
THRESH = 1.0 - 0.01       # halting threshold (1 - EPS)
UNC = 3                   # unconditional (pipelined) leading steps

F32 = dt.float32
F32R = dt.float32r

LAST_EXEC_NS = None
LAST_RESULT = None


def _build(nmax: int):
    nc = bacc.Bacc("TRN2", target_bir_lowering=False, debug=False, num_devices=NCORES)

    # ---- DRAM I/O (per-core shard views; weights replicated) ----
    # fp32r-typed inputs are pre-rounded on the host (RNE, drop 12 bits)
    # [P, KH*BS] SBUF-image layouts: element [p, k*BS+b] = T[k*128+p, b]
    d_xT = nc.dram_tensor("xT", [P, KH * BS], F32R, kind="ExternalInput").ap()
    d_h0T = nc.dram_tensor("h0T", [P, KH * BS], F32R, kind="ExternalInput").ap()
    d_c0T = nc.dram_tensor("c0T", [P, KH * BS], F32, kind="ExternalInput").ap()
    # W_ih[:,1:].T / W_out.T pre-tiled on host into [128, n_m*KH*128] lhsT
    # stream order: block (m, k) at free offset (m*KH + k)*128
    d_wihL = nc.dram_tensor("wihL", [P, 16 * KH * P], F32R, kind="ExternalInput").ap()
    d_woL = nc.dram_tensor("woL", [P, 4 * KH * P], F32R, kind="ExternalInput").ap()
    d_whh = nc.dram_tensor("WhhT", [P, KH * G4], F32R, kind="ExternalInput").ap()  # W_hh.T image
    d_biasv = nc.dram_tensor("biasv", [P, 16], F32, kind="ExternalInput").ap()  # b_ih+b_hh
    d_flagv = nc.dram_tensor("flagv", [P, 16], F32, kind="ExternalInput").ap()  # W_ih[:,0]
    d_boutv = nc.dram_tensor("boutv", [P, 4], F32, kind="ExternalInput").ap()   # b_out
    d_whalt = nc.dram_tensor("whalt", [P, KH], F32R, kind="ExternalInput").ap()  # W_halt.T
    d_bhalt = nc.dram_tensor("bhalt", [1, 1], F32, kind="ExternalInput").ap()
    d_ident = nc.dram_tensor("ident", [P, P], F32R, kind="ExternalInput").ap()

    d_outT = nc.dram_tensor("outT", [P, KH * BS], F32, kind="ExternalOutput").ap()
    d_hfinT = nc.dram_tensor("hfinT", [P, KH * BS], F32, kind="ExternalOutput").ap()
    d_cfinT = nc.dram_tensor("cfinT", [P, KH * BS], F32, kind="ExternalOutput").ap()
    d_pond = nc.dram_tensor("ponder", [1, BS], F32, kind="ExternalOutput").ap()


    with tile.TileContext(nc) as tc:
        with (
            tc.tile_pool(name="wts", bufs=1) as wts,
            tc.tile_pool(name="wl", bufs=2) as wl,   # streamed lhsT tiles (outside Ifs only)
            tc.tile_pool(name="pps", bufs=1, space="PSUM") as pps,
        ):
            # ---------- persistent PSUM tiles (8 banks total) ----------
            gps = [pps.tile([P, 2 * BS], F32, tag=f"gps{j}", name=f"gps{j}")
                   for j in range(3)]
            ps_h = pps.tile([1, BS], F32, tag="ps_h")
            ps_w = pps.tile([P, BS], F32, tag="ps_w")

            # ---------- load weights & state ----------
            ident = wts.tile([P, P], F32R, tag="ident")
            nc.sync.dma_start(ident[:], d_ident)
            # warm the PE clock (HAM) with cheap matmuls while inputs stream
            for i in range(48):
                nc.tensor.matmul(ps_w[:, 0:P], ident[:], ident[:],
                                 start=True, stop=True)
            xt = wts.tile([P, KH * BS], F32R, tag="xt")
            for j in range(2):
                hw = KH * BS // 2
                nc.sync.dma_start(xt[:, j * hw:(j + 1) * hw],
                                  d_xT[:, j * hw:(j + 1) * hw])
            biasv = wts.tile([P, 16], F32, tag="biasv")
            nc.sync.dma_start(biasv[:], d_biasv)
            flagv = wts.tile([P, 16], F32, tag="flagv")
            nc.sync.dma_start(flagv[:], d_flagv)
            boutv = wts.tile([P, 4], F32, tag="boutv")
            nc.sync.dma_start(boutv[:], d_boutv)
            whalt = wts.tile([P, KH], F32R, tag="whalt")
            nc.sync.dma_start(whalt[:], d_whalt)
            bhalt = wts.tile([1, 1], F32, tag="bhalt")
            nc.sync.dma_start(bhalt[:], d_bhalt)
            whh = wts.tile([P, KH * G4], F32R, tag="whh")
            h = wts.tile([P, KH * BS], F32R, tag="h")
            c = wts.tile([P, KH * BS], F32, tag="c")

            # ones row for the broadcast rank-1 matmul (Memset can't write
            # fp32r; round via a DVE copy)
            ones_f = wts.tile([1, P], F32, tag="ones_f")
            nc.vector.memset(ones_f[:], 1.0)
            ones = wts.tile([1, P], F32R, tag="ones")
            nc.vector.tensor_copy(ones[:], ones_f[:])

            # persistent accumulators / scan state / step temps
            hacc = wts.tile([P, KH * BS], F32, tag="hacc")
            cacc = wts.tile([P, KH * BS], F32, tag="cacc")
            nc.vector.memset(hacc[:], 0.0)
            nc.gpsimd.memset(cacc[:], 0.0)
            cum = wts.tile([1, BS], F32, tag="cum")
            nc.vector.memset(cum[:], 0.0)
            pond = wts.tile([1, BS], F32, tag="pond")
            nc.gpsimd.memset(pond[:], 0.0)
            aA = wts.tile([1, BS], F32, tag="aA")
            nc.vector.memset(aA[:], 1.0)
            aB = wts.tile([1, BS], F32, tag="aB")

            sig_i = wts.tile([P, KH * BS], F32, tag="sig_i")
            sig_f = wts.tile([P, KH * BS], F32, tag="sig_f")
            tg = wts.tile([P, KH * BS], F32, tag="tg")
            sig_o = wts.tile([P, KH * BS], F32, tag="sig_o")
            halt = wts.tile([1, BS], F32, tag="halt")
            u1 = wts.tile([1, BS], F32, tag="u1")
            tuc = wts.tile([1, BS], F32, tag="tuc")
            w = wts.tile([1, BS], F32, tag="w")
            pc = wts.tile([1, BS], F32, tag="pc")
            wr = wts.tile([1, BS], F32R, tag="wr")
            wbc4 = wts.tile([P, KH * BS], F32, tag="wbc4")
            hacc_r32 = wts.tile([P, KH * BS], F32, tag="hacc_r32")
            nact = wts.tile([1, 1], F32, tag="nact")
            nacti = wts.tile([1, 1], dt.int32, tag="nacti")
            outsb = wts.tile([P, KH * BS], F32, tag="outsb")

            # ---------- x_part = W_ih' @ x.T + (b_ih + b_hh) ----------
            # evacuate via DVE tensor_scalar (adds the per-partition bias and
            # rounds to fp32r in one op)
            xp = wts.tile([P, 16 * BS], F32R, tag="xp")
            wlt4 = None
            for g in range(8):
                ps = gps[g % 3]
                for half in range(2):
                    m = 2 * g + half
                    sl = ps[:, half * BS:(half + 1) * BS]
                    if m % 4 == 0:
                        wlt4 = wl.tile([P, 4 * KH * P], F32R, tag="wl",
                                       name=f"wih_m{m}")
                        nc.sync.dma_start(
                            wlt4[:], d_wihL[:, m * KH * P:(m + 4) * KH * P])
                    mo = (m % 4) * KH * P
                    for k in range(KH):
                        nc.tensor.matmul(
                            sl,
                            wlt4[:, mo + k * P: mo + (k + 1) * P],
                            xt[:, k * BS:(k + 1) * BS],
                            start=(k == 0), stop=(k == KH - 1),
                        )
                    nc.vector.tensor_scalar(xp[:, m * BS:(m + 1) * BS], sl,
                                            biasv[:, m:m + 1], None, Alu.add)

            # bulk weights/state stream in behind the prologue's inputs
            for k in range(KH):
                nc.sync.dma_start(whh[:, k * G4:(k + 1) * G4],
                                  d_whh[:, k * G4:(k + 1) * G4])
            for j in range(2):
                hw = KH * BS // 2
                nc.sync.dma_start(h[:, j * hw:(j + 1) * hw],
                                  d_h0T[:, j * hw:(j + 1) * hw])
                nc.sync.dma_start(c[:, j * hw:(j + 1) * hw],
                                  d_c0T[:, j * hw:(j + 1) * hw])

            gate_dst = [(sig_i, Act.Sigmoid), (sig_f, Act.Sigmoid),
                        (tg, Act.Tanh), (sig_o, Act.Sigmoid)]

            # ---------- one ACT-LSTM step ----------
            def make_acc(n):
                """Emit step n's w-broadcast + weighted accumulation.

                Deferred into the next step's matmul burst (or the epilogue)
                so the in-order PE queue never stalls on the small-vector
                chain. Reads h/c before the next cell update rewrites them
                (Tile's WAR deps order that)."""
                def emit():
                    nc.tensor.matmul(ps_w[:], ones[:], wr[:], start=True, stop=True)
                    for k in range(KH):
                        ck = slice(k * BS, (k + 1) * BS)
                        nc.scalar.copy(wbc4[:, ck], ps_w[:])
                        # outsb is idle until the epilogue; use as the c temp,
                        # then fold h into wbc4 in place (dead afterwards)
                        nc.vector.tensor_tensor(outsb[:, ck], c[:, ck],
                                                wbc4[:, ck], Alu.mult)
                        nc.gpsimd.tensor_tensor(cacc[:, ck], cacc[:, ck],
                                                outsb[:, ck], Alu.add)
                        nc.vector.tensor_tensor(wbc4[:, ck], h[:, ck].bitcast(F32),
                                                wbc4[:, ck], Alu.mult)
                        nc.vector.tensor_tensor(hacc[:, ck], hacc[:, ck],
                                                wbc4[:, ck], Alu.add)
                return emit

            def step(n: int, emit_prev_acc):
                first = n == 0
                last = n == nmax - 1
                a_cur = aA if n % 2 == 0 else aB
                a_nxt = aB if n % 2 == 0 else aA

                # front-run the first 3 groups' x_part injections (no h dep),
                # then the halt matvec of the PREVIOUS h... the halt for THIS
                # step comes after the cell below.
                for g in range(3):
                    ps = gps[g]
                    for half in range(2):
                        m = 2 * g + half
                        nc.tensor.matmul(ps[:, half * BS:(half + 1) * BS],
                                         ident[:], xp[:, m * BS:(m + 1) * BS],
                                         start=True, stop=False)

                # gates.T = x_part.T + W_hh @ h.T ; evacuate through the
                # activation fns (step 0 adds the ACT flag column as bias)
                for g in range(8):
                    ps = gps[g % 3]
                    for half in range(2):
                        m = 2 * g + half
                        sl = ps[:, half * BS:(half + 1) * BS]
                        if g >= 3:
                            nc.tensor.matmul(sl, ident[:],
                                             xp[:, m * BS:(m + 1) * BS],
                                             start=True, stop=False)
                        for k in range(KH):
                            nc.tensor.matmul(
                                sl,
                                whh[:, k * G4 + m * P: k * G4 + (m + 1) * P],
                                h[:, k * BS:(k + 1) * BS],
                                start=False, stop=(k == KH - 1),
                            )
                    dst, fn = gate_dst[g // 2]
                    off = (g % 2) * 2 * BS
                    if first:
                        for half in range(2):
                            m = 2 * g + half
                            nc.scalar.activation(
                                dst[:, off + half * BS: off + (half + 1) * BS],
                                ps[:, half * BS:(half + 1) * BS], fn,
                                bias=flagv[:, m:m + 1])
                    else:
                        nc.scalar.activation(dst[:, off:off + 2 * BS], ps[:], fn)
                    if g == 1 and emit_prev_acc is not None:
                        emit_prev_acc()

                # LSTM cell, chunked so h[k] lands early (shortens the
                # loop-carried path into the next step's matmuls)
                for k in range(KH):
                    ck = slice(k * BS, (k + 1) * BS)
                    nc.vector.tensor_tensor(tg[:, ck], sig_i[:, ck], tg[:, ck], Alu.mult)
                    nc.vector.tensor_tensor(sig_f[:, ck], sig_f[:, ck], c[:, ck], Alu.mult)
                    nc.vector.tensor_tensor(c[:, ck], sig_f[:, ck], tg[:, ck], Alu.add)
                    nc.scalar.activation(tg[:, ck], c[:, ck], Act.Tanh)
                    nc.vector.tensor_tensor(h[:, ck], sig_o[:, ck], tg[:, ck], Alu.mult)
                    # halt matvec chunk as soon as h[k] exists
                    nc.tensor.matmul(ps_h[:], whalt[:, k:k + 1], h[:, ck],
                                     start=(k == 0), stop=(k == KH - 1))
                nc.scalar.activation(halt[:], ps_h[:], Act.Sigmoid,
                                     bias=bhalt[0:1, 0:1])

                # ACT weights:  w = a * (halt + (cum>=t) * (1 - cum_new))
                nc.vector.tensor_tensor(cum[:], cum[:], halt[:], Alu.add)
                if not last:
                    # exit mask first: the step-(n+1) If waits on this
                    nc.vector.tensor_scalar(a_nxt[:], cum[:], THRESH, 0.0,
                                            Alu.is_lt, Alu.add, accum_out=nact[:])
                    nc.vector.tensor_copy(nacti[:], nact[:])
                nc.vector.tensor_scalar(u1[:], cum[:], -1.0, 1.0, Alu.mult, Alu.add)
                cf_thresh = -3.0e38 if last else THRESH
                nc.vector.scalar_tensor_tensor(tuc[:], cum[:], cf_thresh, u1[:],
                                               Alu.is_ge, Alu.mult)
                nc.vector.scalar_tensor_tensor(w[:], tuc[:], 0.0, halt[:],
                                               Alu.bypass, Alu.add)
                nc.vector.scalar_tensor_tensor(w[:], w[:], 0.0, a_cur[:],
                                               Alu.bypass, Alu.mult)
                nc.vector.tensor_copy(wr[:], w[:])

                # ponder += a + (cum>=t)*w   (off the w critical path)
                nc.vector.scalar_tensor_tensor(pc[:], cum[:], cf_thresh, w[:],
                                               Alu.is_ge, Alu.mult)
                nc.vector.scalar_tensor_tensor(pc[:], pc[:], 0.0, a_cur[:],
                                               Alu.bypass, Alu.add)
                nc.gpsimd.tensor_tensor(pond[:], pond[:], pc[:], Alu.add)


            def run_steps(n: int, emit_prev_acc):
                step(n, emit_prev_acc)
                acc_self = make_acc(n)
                if n + 1 >= nmax:
                    acc_self()
                    return
                if n + 1 < UNC:
                    run_steps(n + 1, acc_self)
                    return
                v = nc.values_load(nacti[:])
                with tc.If(v > 0) as cmp:
                    run_steps(n + 1, acc_self)
                with cmp.Else():
                    acc_self()

            run_steps(0, None)

            # ---------- epilogue: out.T = W_out @ h_fin + b_out ----------
            for j in range(2):
                hw = KH * BS // 2
                nc.sync.dma_start(d_hfinT[:, j * hw:(j + 1) * hw],
                                  hacc[:, j * hw:(j + 1) * hw])
                nc.sync.dma_start(d_cfinT[:, j * hw:(j + 1) * hw],
                                  cacc[:, j * hw:(j + 1) * hw])
            nc.sync.dma_start(d_pond, pond[:])
            # chunked fp32r rounding of h_fin so the projection can start
            # as soon as chunk 0 is ready
            hacc_r = hacc_r32[:].bitcast(F32R)
            for k in range(KH):
                ck = slice(k * BS, (k + 1) * BS)
                nc.vector.tensor_copy(hacc_r32[:, ck].bitcast(F32R), hacc[:, ck])
            wlo = wl.tile([P, 4 * KH * P], F32R, tag="wl", name="wo_all")
            nc.sync.dma_start(wlo[:], d_woL[:])
            for g in range(2):
                ps = gps[g % 3]
                for half in range(2):
                    m = 2 * g + half
                    sl = ps[:, half * BS:(half + 1) * BS]
                    mo = m * KH * P
                    for k in range(KH):
                        nc.tensor.matmul(
                            sl,
                            wlo[:, mo + k * P: mo + (k + 1) * P],
                            hacc_r32[:, k * BS:(k + 1) * BS].bitcast(F32R),
                            start=(k == 0), stop=(k == KH - 1),
                        )
                    nc.scalar.activation(outsb[:, m * BS:(m + 1) * BS], sl,
                                         Act.Identity, bias=boutv[:, m:m + 1])

            for j in range(2):
                hw = KH * BS // 2
                nc.sync.dma_start(d_outT[:, j * hw:(j + 1) * hw],
                                  outsb[:, j * hw:(j + 1) * hw])

    nc.compile()
    return nc


_CACHE: dict = {}


def _get_nc(nmax: int):
    if nmax not in _CACHE:
        _CACHE[nmax] = _build(nmax)
    return _CACHE[nmax]


def _r(x: np.ndarray) -> np.ndarray:
    """Round fp32 -> fp32r bits (RNE dropping the low 12 mantissa bits)."""
    shape = x.shape
    u = np.ascontiguousarray(x, np.float32).view(np.uint32).astype(np.uint64)
    mask = np.uint64(0xFFF)
    half = np.uint64(0x800)
    low = u & mask
    base = u & ~mask
    lsb = (u >> np.uint64(12)) & np.uint64(1)
    add = (low > half) | ((low == half) & (lsb == np.uint64(1)))
    out = base + np.where(add, np.uint64(0x1000), np.uint64(0))
    return (out & np.uint64(0xFFFFFFFF)).astype(np.uint32).view(np.float32).reshape(shape)


def _img(T: np.ndarray) -> np.ndarray:
    """[KH*128, N] -> SBUF image [128, KH*N]: img[p, k*N+n] = T[k*128+p, n]."""
    kh = T.shape[0] // P
    return np.ascontiguousarray(
        T.reshape(kh, P, T.shape[1]).transpose(1, 0, 2).reshape(P, kh * T.shape[1]))


def _unimg(img: np.ndarray, kh: int) -> np.ndarray:
    """Inverse of _img: [128, kh*N] -> [kh*128, N]."""
    N = img.shape[1] // kh
    return img.reshape(P, kh, N).transpose(1, 0, 2).reshape(kh * P, N)


def _lhsT_stream(WT: np.ndarray, n_m: int) -> np.ndarray:
    """[K, M] -> [128, n_m*KH*128] with block (m, k) at offset (m*KH+k)*128.

    Element [p, (m*KH+k)*128 + q] = WT[k*128 + p, m*128 + q].
    """
    K, M = WT.shape
    kh = K // P
    assert M == n_m * P
    t = WT.reshape(kh, P, n_m, P).transpose(1, 2, 0, 3).reshape(P, n_m * kh * P)
    return np.ascontiguousarray(t)


def kernel(x, h0, c0, W_ih, b_ih, W_hh, b_hh, W_halt, b_halt, W_out, b_out,
           max_steps):
    global LAST_EXEC_NS, LAST_RESULT
    f = np.float32
    x = np.asarray(x, f)
    h0 = np.asarray(h0, f)
    c0 = np.asarray(c0, f)
    W_ih = np.asarray(W_ih, f)
    b_ih = np.asarray(b_ih, f)
    W_hh = np.asarray(W_hh, f)
    b_hh = np.asarray(b_hh, f)
    W_halt = np.asarray(W_halt, f)
    b_halt = np.asarray(b_halt, f)
    W_out = np.asarray(W_out, f)
    b_out = np.asarray(b_out, f)
    nmax = int(max_steps)

    shared = {
        "wihL": _r(_lhsT_stream(np.ascontiguousarray(W_ih[:, 1:].T), 16)),
        "woL": _r(_lhsT_stream(np.ascontiguousarray(W_out.T), 4)),
        "WhhT": _r(_img(W_hh.T)),
        "biasv": np.ascontiguousarray((b_ih + b_hh).reshape(16, P).T),
        "flagv": np.ascontiguousarray(W_ih[:, 0].reshape(16, P).T),
        "boutv": np.ascontiguousarray(b_out.reshape(4, P).T),
        "whalt": _r(np.ascontiguousarray(W_halt[0].reshape(KH, P).T)),
        "bhalt": b_halt.reshape(1, 1),
        "ident": np.eye(P, dtype=f),
    }
    in_maps = []
    for i in range(NCORES):
        s = slice(i * BS, (i + 1) * BS)
        in_maps.append({
            "xT": _r(_img(x[s].T)),
            "h0T": _r(_img(h0[s].T)),
            "c0T": _img(c0[s].T),
            **shared,
        })

    nc = _get_nc(nmax)
    res = run_bass_kernel_spmd(nc, in_maps, core_ids=list(range(NCORES)))
    LAST_EXEC_NS = res.exec_time_ns
    LAST_RESULT = res

    outs, hfs, cfs, ponds = [], [], [], []
    for i in range(NCORES):
        r = res.results[i]
        outs.append(_unimg(r["outT"], KH).T)
        hfs.append(_unimg(r["hfinT"], KH).T)
        cfs.append(_unimg(r["cfinT"], KH).T)
        ponds.append(r["ponder"][0])
    output = np.ascontiguousarray(np.concatenate(outs, 0))
    h_fin = np.ascontiguousarray(np.concatenate(hfs, 0))
    c_fin = np.ascontiguousarray(np.concatenate(cfs, 0))
    ponder = np.ascontiguousarray(np.concatenate(ponds, 0))
    return output, h_fin, c_fin, ponder
